# revision 21
# baseline (speedup 1.0000x reference)
"""Trainium2 Bass kernel for nn_AttentionBlock (B=2, S=2048, D=1024, H=16).

Sharding: 8 cores = data-parallel over batch (2) x tensor-parallel over
head groups (4 heads per core).  Each core computes its 4 heads'
attention plus its slice of the qkv / out projections; the host sums the
4 per-batch partial outputs and adds b_out.

Per-core layout plan (all matmuls in float32r, N>=256):
  - host passes x[b].T so the contraction dim (d) is the partition dim
  - q,k computed transposed [e, s]; v computed natural [s, hd]
  - S^T[j,i] = k_h q_h^T, two heads packed in the PE array (row groups)
  - exp on ScalarE straight out of PSUM (scale = 1/8 folded into exp)
  - PV matmul with stationary [v_h | ones] -> unnormalized out^T plus the
    softmax row-sum in PSUM row 64, in one pass over E
  - normalize: reciprocal + gpsimd partition_broadcast + DVE multiply
  - final projection consumes the transposed attention output directly
"""

from contextlib import ExitStack
from functools import partial

import numpy as np

import concourse.bass as bass
import concourse.tile as tile
from concourse import bacc, mybir
from concourse import bass_utils

B, S, D = 2, 2048, 1024
HD = 64          # head dim
HPC = 4          # heads per core
E_QK = 512       # q+k columns per core (2 * HPC * HD)
E_V = 256        # v columns per core
NCORES = 8

F32 = mybir.dt.float32
F32R = mybir.dt.float32r

S_TILES = S // 128       # 16
D_TILES = D // 128       # 8
I_CHUNKS = S // 512      # 4 query chunks
J_TILES = S // 128       # 16 key tiles


def _build_nc(reps=1):
    nc = bacc.Bacc("TRN2", target_bir_lowering=False, debug=False, num_devices=NCORES)

    xT = nc.dram_tensor("xT", [D, S], F32R, kind="ExternalInput")
    w_qk = nc.dram_tensor("w_qk", [D, E_QK], F32R, kind="ExternalInput")
    w_v = nc.dram_tensor("w_v", [D, E_V], F32R, kind="ExternalInput")
    w_o = nc.dram_tensor("w_o", [E_V, D], F32R, kind="ExternalInput")
    b_qk = nc.dram_tensor("b_qk", [128, 4], F32, kind="ExternalInput")
    b_v = nc.dram_tensor("b_v", [1, E_V], F32R, kind="ExternalInput")
    ones = nc.dram_tensor("ones", [128, 128], F32R, kind="ExternalInput")
    out = nc.dram_tensor("out", [S, D], F32, kind="ExternalOutput")

    with tile.TileContext(nc) as tc, ExitStack() as ctx:
        if reps == 1:
            _body(ctx, tc, xT.ap(), w_qk.ap(), w_v.ap(), w_o.ap(), b_qk.ap(), b_v.ap(), ones.ap(), out.ap())
        else:
            with tc.For_i(0, reps) as _i:
                with ExitStack() as ictx:
                    _body(ictx, tc, xT.ap(), w_qk.ap(), w_v.ap(), w_o.ap(), b_qk.ap(), b_v.ap(), ones.ap(), out.ap())
    nc.compile()
    return nc


def _body(ctx, tc, xT, w_qk, w_v, w_o, b_qk, b_v, ones, out):
    nc = tc.nc
    Exp = mybir.ActivationFunctionType.Exp

    persist = ctx.enter_context(tc.tile_pool(name="persist", bufs=1))
    ps_s = ctx.enter_context(tc.tile_pool(name="ps_s", bufs=2, space="PSUM"))
    ps_q = ctx.enter_context(tc.tile_pool(name="ps_q", bufs=2, space="PSUM"))
    ps_pv = ctx.enter_context(tc.tile_pool(name="ps_pv", bufs=2, space="PSUM"))
    epool = ctx.enter_context(tc.tile_pool(name="epool", bufs=6))
    spool = ctx.enter_context(tc.tile_pool(name="spool", bufs=3))
    rpool = ctx.enter_context(tc.tile_pool(name="rpool", bufs=4))

    # ---- persistent SBUF tensors ----
    xT_sb = persist.tile([128, D_TILES, S], F32R, name="xT_sb")
    w_qk_sb = persist.tile([128, D_TILES, E_QK], F32R, name="w_qk_sb")
    w_v_sb = persist.tile([128, D_TILES, E_V], F32R, name="w_v_sb")
    w_o_sb = persist.tile([128, 2, D], F32R, name="w_o_sb")
    b_qk_sb = persist.tile([128, 4], F32, name="b_qk_sb")
    b_v_sb = persist.tile([1, E_V], F32R, name="b_v_sb")
    ones_sb = persist.tile([1, 128], F32R, name="ones_sb")
    qkT_sb = persist.tile([128, 4, S], F32R, name="qkT_sb")   # tiles 0-1: qT, 2-3: kT
    v_sb = persist.tile([128, S_TILES, HPC, HD + 1], F32R, name="v_sb")
    attnT_sb = persist.tile([128, 2, S], F32R, name="attnT_sb")

    # ---- input DMAs, ordered so the first attention chain's data lands
    # first: xT s-chunk 0 + the pair-0 q,k weight columns, then the rest ----
    dsl = lambda t: slice(t * 128, (t + 1) * 128)
    for t in range(D_TILES):
        nc.sync.dma_start(xT_sb[:, t, 0:512], xT[dsl(t), 0:512])
        nc.sync.dma_start(w_qk_sb[:, t, 0:384], w_qk[dsl(t), 0:384])  # q + k pair0 (+q pair1)
    nc.sync.dma_start(b_qk_sb[:], b_qk[:, :])
    nc.sync.dma_start(b_v_sb[:], b_v[:, :])
    nc.sync.dma_start(ones_sb[:], ones[0:1, 0:128])
    nc.sync.dma_start(v_sb[:, :, :, HD], ones[:, 0:64].rearrange("p (s h) -> p s h", s=S_TILES))
    for t in range(D_TILES):
        nc.sync.dma_start(w_v_sb[:, t, :], w_v[dsl(t), :])
    for sc in range(1, I_CHUNKS):
        for t in range(D_TILES):
            nc.sync.dma_start(xT_sb[:, t, sc * 512:(sc + 1) * 512],
                              xT[dsl(t), sc * 512:(sc + 1) * 512])
    for t in range(D_TILES):
        nc.sync.dma_start(w_qk_sb[:, t, 384:512], w_qk[dsl(t), 384:512])  # k pair1
    for t in range(2):
        nc.sync.dma_start(w_o_sb[:, t, :], w_o[dsl(t), :])

    # ---- projection emitters ----
    def emit_qk(et, sc):
        psum = ps_q.tile([128, 512], F32, name="ps_qk", tag="psq")
        for d in range(D_TILES):
            nc.tensor.matmul(
                psum,
                (w_qk_sb[:, d, et * 128:(et + 1) * 128]),
                (xT_sb[:, d, sc * 512:(sc + 1) * 512]),
                start=(d == 0), stop=(d == D_TILES - 1),
            )
        nc.vector.tensor_scalar_add(
            qkT_sb[:, et, sc * 512:(sc + 1) * 512], psum, b_qk_sb[:, et:et + 1],
        )

    def emit_v():
        for st in range(S_TILES):
            psum = ps_q.tile([128, 512], F32, name="ps_v", tag="psq")[:, :E_V]
            for d in range(D_TILES):
                nc.tensor.matmul(
                    psum,
                    (xT_sb[:, d, st * 128:(st + 1) * 128]),
                    (w_v_sb[:, d, :]),
                    start=(d == 0), stop=False,
                )
            # bias via rank-1 ones matmul (K=1)
            nc.tensor.matmul(psum, (ones_sb[:, :]), (b_v_sb[:, :]), start=False, stop=True)
            nc.vector.tensor_copy(
                v_sb[:, st, :, 0:HD],
                psum.rearrange("p (h e) -> p h e", h=HPC),
            )

    def emit_attention(ic, pair):
        if True:
            isl = slice(ic * 512, (ic + 1) * 512)
            pvA = ps_pv.tile([HD + 1, 512], F32, name="pvA", tag="pv")
            pvB = ps_pv.tile([HD + 1, 512], F32, name="pvB", tag="pv")
            for j in range(J_TILES):
                jsl = slice(j * 128, (j + 1) * 128)
                psS = ps_s.tile([128, 1024], F32, name="psS", tag="pss")
                nc.tensor.matmul(
                    psS[:, 0:512],
                    (qkT_sb[0:64, 2 + pair, jsl]),
                    (qkT_sb[0:64, pair, isl]),
                    start=True, stop=True, tile_position=(0, 0),
                )
                nc.tensor.matmul(
                    psS[:, 512:1024],
                    (qkT_sb[64:128, 2 + pair, jsl]),
                    (qkT_sb[64:128, pair, isl]),
                    start=True, stop=True, tile_position=(64, 0),
                )
                e_t = epool.tile([128, 1024], F32R, name="e_t")
                nc.scalar.activation(e_t[:], psS[:], Exp, scale=0.125)
                nc.tensor.matmul(
                    pvA[:], (v_sb[:, j, 2 * pair, :]), (e_t[:, 0:512]),
                    start=(j == 0), stop=(j == J_TILES - 1),
                )
                nc.tensor.matmul(
                    pvB[:], (v_sb[:, j, 2 * pair + 1, :]), (e_t[:, 512:1024]),
                    start=(j == 0), stop=(j == J_TILES - 1),
                )
            for h_loc, pv in ((0, pvA), (1, pvB)):
                rec = rpool.tile([1, 512], F32, name="rec", tag="rec")
                nc.vector.reciprocal(rec[:], pv[HD:HD + 1, :])
                rb = rpool.tile([HD, 512], F32, name="rb", tag="rb")
                nc.gpsimd.partition_broadcast(rb[:], rec[:])
                nc.vector.tensor_mul(
                    attnT_sb[h_loc * 64:(h_loc + 1) * 64, pair, isl],
                    pv[0:HD, :],
                    rb[:],
                )

    def emit_final(ic):
        # final projection for the 4 s-tiles of this query chunk
        for st in range(ic * 4, ic * 4 + 4):
            ssl = slice(st * 128, (st + 1) * 128)
            for mc in range(2):
                msl = slice(mc * 512, (mc + 1) * 512)
                psF = ps_q.tile([128, 512], F32, name="ps_f", tag="psq")
                for kk in range(2):
                    nc.tensor.matmul(
                        psF,
                        (attnT_sb[:, kk, ssl]),
                        (w_o_sb[:, kk, msl]),
                        start=(kk == 0), stop=(kk == 1),
                    )
                o_t = spool.tile([128, 512], F32, name="o_t")
                nc.vector.tensor_copy(o_t[:], psF)
                nc.sync.dma_start(out[ssl, msl], o_t[:])

    # ---- emission order: pair-0 attention starts as early as possible so
    # the Activation engine (softmax exp, the co-bottleneck) fills while the
    # PE is still on projections ----
    # pair-0 q,k projections sc-major so the first attention chain's two
    # bias-adds are the first two DVE ops
    for sc in range(I_CHUNKS):
        emit_qk(2, sc)    # k pair 0
        emit_qk(0, sc)    # q pair 0
    emit_v()
    emit_attention(0, 0)
    for sc in range(I_CHUNKS):
        emit_qk(3, sc)    # k pair 1 (PE filler while ACT chews on ic0/pair0)
        emit_qk(1, sc)    # q pair 1
    emit_attention(0, 1)
    for ic in range(1, I_CHUNKS):
        emit_attention(ic, 0)
        emit_final(ic - 1)   # previous chunk's projection: PE filler
        emit_attention(ic, 1)
    emit_final(I_CHUNKS - 1)


_CACHE = {}


def _get_nc(reps=1):
    key = ("nc", reps)
    if key not in _CACHE:
        _CACHE[key] = _build_nc(reps)
    return _CACHE[key]


def _get_runner(reps=1):
    """Build (once) a jitted shard_map executable over the 8 cores.

    Mirrors bass2jax.run_bass_via_pjrt but caches the jitted function so
    repeat kernel() calls and benchmarking skip retrace/recompile.
    """
    if ("runner", reps) in _CACHE:
        return _CACHE[("runner", reps)]
    import jax
    import jax.numpy as jnp
    from jax.sharding import Mesh, PartitionSpec
    from jax.experimental.shard_map import shard_map
    from concourse import bass2jax

    nc = _get_nc(reps)
    bass2jax.install_neuronx_cc_hook()

    partition_name = nc.partition_id_tensor.name if nc.partition_id_tensor else None
    in_names, out_names, out_avals = [], [], []
    for alloc in nc.m.functions[0].allocations:
        if not isinstance(alloc, mybir.MemoryLocationSet):
            continue
        name = alloc.memorylocations[0].name
        if alloc.kind == "ExternalInput":
            if name != partition_name:
                in_names.append(name)
        elif alloc.kind == "ExternalOutput":
            shape = tuple(alloc.tensor_shape)
            dtype = mybir.dt.np(alloc.dtype)
            out_names.append(name)
            out_avals.append(jax.core.ShapedArray(shape, dtype))
    n_params = len(in_names)
    n_outs = len(out_avals)
    all_names = in_names + out_names
    if partition_name is not None:
        all_names = all_names + [partition_name]
    donate = tuple(range(n_params, n_params + n_outs))

    def _jit_body(*args):
        operands = list(args)
        if partition_name is not None:
            operands.append(bass2jax.partition_id_tensor())
        outs = bass2jax._bass_exec_p.bind(
            *operands,
            out_avals=tuple(out_avals),
            in_names=tuple(all_names),
            out_names=tuple(out_names),
            lowering_input_output_aliases=(),
            sim_require_finite=True,
            sim_require_nnan=True,
            nc=nc,
        )
        return tuple(outs)

    devices = jax.devices()[:NCORES]
    mesh = Mesh(np.asarray(devices), ("core",))
    sharded = jax.jit(
        shard_map(
            _jit_body, mesh=mesh,
            in_specs=(PartitionSpec("core"),) * (n_params + n_outs),
            out_specs=(PartitionSpec("core"),) * n_outs,
            check_rep=False,
        ),
        donate_argnums=donate, keep_unused=True,
    )

    from jax.sharding import NamedSharding
    core_sharding = NamedSharding(mesh, PartitionSpec("core"))

    @partial(jax.jit, out_shardings=core_sharding)
    def _zeros():
        return tuple(
            jnp.zeros((NCORES * a.shape[0],) + a.shape[1:], a.dtype)
            for a in out_avals)

    def run(in_maps, device_arrays=None, timeit=False):
        import time as _time
        if device_arrays is None:
            concat_in = [
                np.concatenate([np.asarray(m[name]) for m in in_maps], axis=0)
                for name in in_names]
            device_arrays = [jax.device_put(a, core_sharding) for a in concat_in]
        zs = jax.block_until_ready(_zeros())
        t0 = _time.perf_counter()
        out_arrs = jax.block_until_ready(sharded(*device_arrays, *zs))
        dt = _time.perf_counter() - t0
        results = [
            {name: np.asarray(out_arrs[i]).reshape(NCORES, *out_avals[i].shape)[c]
             for i, name in enumerate(out_names)}
            for c in range(NCORES)]
        if timeit:
            return results, dt, device_arrays
        return results

    def bench(in_maps, iters=10, batches=3):
        """Pipelined timing: dispatch `iters` executions back-to-back and
        block once, amortizing the per-dispatch RPC latency.  Returns the
        min per-iter average across `batches` batches."""
        import time as _time
        concat_in = [
            np.concatenate([np.asarray(m[name]) for m in in_maps], axis=0)
            for name in in_names]
        device_arrays = [jax.device_put(a, core_sharding) for a in concat_in]
        best = None
        for b in range(batches):
            all_zs = [jax.block_until_ready(_zeros()) for _ in range(iters + 1)]
            jax.block_until_ready(sharded(*device_arrays, *all_zs[0]))
            t0 = _time.perf_counter()
            outs = [sharded(*device_arrays, *all_zs[1 + i]) for i in range(iters)]
            jax.block_until_ready(outs)
            dt = (_time.perf_counter() - t0) / iters
            best = dt if best is None else min(best, dt)
        return best

    _CACHE[("bench", reps)] = bench
    _CACHE[("runner", reps)] = run
    return run


def _core_inputs(x, w_qkv, b_qkv, w_out):
    """Host-side sharding: returns the 8 per-core input dicts."""
    in_maps = []
    for c in range(NCORES):
        b, g = divmod(c, 4)
        e0 = g * HPC * HD  # first column of this core's head group
        q_cols = slice(e0, e0 + E_V)
        k_cols = slice(D + e0, D + e0 + E_V)
        v_cols = slice(2 * D + e0, 2 * D + e0 + E_V)
        w_qk_c = np.ascontiguousarray(
            np.concatenate([w_qkv[:, q_cols], w_qkv[:, k_cols]], axis=1))
        b_qk_c = np.ascontiguousarray(
            np.concatenate([b_qkv[q_cols], b_qkv[k_cols]]).reshape(4, 128).T)
        in_maps.append({
            "xT": np.ascontiguousarray(x[b].T),
            "w_qk": w_qk_c,
            "w_v": np.ascontiguousarray(w_qkv[:, v_cols]),
            "w_o": np.ascontiguousarray(w_out[e0:e0 + E_V, :]),
            "b_qk": b_qk_c,
            "b_v": np.ascontiguousarray(b_qkv[v_cols]).reshape(1, E_V),
            "ones": np.ones((128, 128), dtype=np.float32),
        })
    return in_maps


def kernel(x, w_qkv, b_qkv, w_out, b_out):
    x = np.asarray(x, dtype=np.float32)
    w_qkv = np.asarray(w_qkv, dtype=np.float32)
    b_qkv = np.asarray(b_qkv, dtype=np.float32)
    w_out = np.asarray(w_out, dtype=np.float32)
    b_out = np.asarray(b_out, dtype=np.float32)

    run = _get_runner()
    in_maps = _core_inputs(x, w_qkv, b_qkv, w_out)
    results = run(in_maps)
    partials = np.stack([results[c]["out"] for c in range(NCORES)])
    full = partials.reshape(B, 4, S, D).sum(axis=1) + b_out
    return full.astype(np.float32)


# revision 23
# speedup vs baseline: 1.0310x; 1.0310x over previous
"""Trainium2 Bass kernel for nn_AttentionBlock (B=2, S=2048, D=1024, H=16).

Sharding: 8 cores = data-parallel over batch (2) x tensor-parallel over
head groups (4 heads per core).  Each core computes its 4 heads'
attention plus its slice of the qkv / out projections; the host sums the
4 per-batch partial outputs and adds b_out.

Per-core layout plan (all matmuls in float32r, N>=256):
  - host passes x[b].T so the contraction dim (d) is the partition dim
  - q,k computed transposed [e, s]; v computed natural [s, hd]
  - S^T[j,i] = k_h q_h^T, two heads packed in the PE array (row groups)
  - exp on ScalarE straight out of PSUM (scale = 1/8 folded into exp)
  - PV matmul with stationary [v_h | ones] -> unnormalized out^T plus the
    softmax row-sum in PSUM row 64, in one pass over E
  - normalize: reciprocal + gpsimd partition_broadcast + DVE multiply
  - final projection consumes the transposed attention output directly
"""

from contextlib import ExitStack
from functools import partial

import ml_dtypes
import numpy as np

import concourse.bass as bass
import concourse.tile as tile
from concourse import bacc, mybir
from concourse import bass_utils

B, S, D = 2, 2048, 1024
HD = 64          # head dim
HPC = 4          # heads per core
E_QK = 512       # q+k columns per core (2 * HPC * HD)
E_V = 256        # v columns per core
NCORES = 8

F32 = mybir.dt.float32
F32R = mybir.dt.float32r
BF16 = mybir.dt.bfloat16

S_TILES = S // 128       # 16
D_TILES = D // 128       # 8
I_CHUNKS = S // 512      # 4 query chunks
J_TILES = S // 128       # 16 key tiles


def _build_nc(reps=1):
    nc = bacc.Bacc("TRN2", target_bir_lowering=False, debug=False, num_devices=NCORES)

    xT = nc.dram_tensor("xT", [D, S], F32R, kind="ExternalInput")
    w_qk = nc.dram_tensor("w_qk", [D, E_QK], F32R, kind="ExternalInput")
    w_v = nc.dram_tensor("w_v", [D, E_V], F32R, kind="ExternalInput")
    w_o = nc.dram_tensor("w_o", [E_V, D], F32R, kind="ExternalInput")
    b_qk = nc.dram_tensor("b_qk", [128, 4], F32, kind="ExternalInput")
    b_v = nc.dram_tensor("b_v", [1, E_V], F32R, kind="ExternalInput")
    ones = nc.dram_tensor("ones", [128, 128], F32R, kind="ExternalInput")
    ones16 = nc.dram_tensor("ones16", [128, 64], BF16, kind="ExternalInput")
    out = nc.dram_tensor("out", [S, D], F32, kind="ExternalOutput")

    with tile.TileContext(nc) as tc, ExitStack() as ctx:
        if reps == 1:
            _body(ctx, tc, xT.ap(), w_qk.ap(), w_v.ap(), w_o.ap(), b_qk.ap(), b_v.ap(), ones.ap(), ones16.ap(), out.ap())
        else:
            with tc.For_i(0, reps) as _i:
                with ExitStack() as ictx:
                    _body(ictx, tc, xT.ap(), w_qk.ap(), w_v.ap(), w_o.ap(), b_qk.ap(), b_v.ap(), ones.ap(), ones16.ap(), out.ap())
    nc.compile()
    return nc


def _body(ctx, tc, xT, w_qk, w_v, w_o, b_qk, b_v, ones, ones16, out):
    nc = tc.nc
    Exp = mybir.ActivationFunctionType.Exp

    persist = ctx.enter_context(tc.tile_pool(name="persist", bufs=1))
    ps_s = ctx.enter_context(tc.tile_pool(name="ps_s", bufs=2, space="PSUM"))
    ps_q = ctx.enter_context(tc.tile_pool(name="ps_q", bufs=2, space="PSUM"))
    ps_pv = ctx.enter_context(tc.tile_pool(name="ps_pv", bufs=2, space="PSUM"))
    epool = ctx.enter_context(tc.tile_pool(name="epool", bufs=6))
    spool = ctx.enter_context(tc.tile_pool(name="spool", bufs=3))
    rpool = ctx.enter_context(tc.tile_pool(name="rpool", bufs=4))

    # ---- persistent SBUF tensors ----
    xT_sb = persist.tile([128, D_TILES, S], F32R, name="xT_sb")
    w_qk_sb = persist.tile([128, D_TILES, E_QK], F32R, name="w_qk_sb")
    w_v_sb = persist.tile([128, D_TILES, E_V], F32R, name="w_v_sb")
    w_o_sb = persist.tile([128, 2, D], F32R, name="w_o_sb")
    b_qk_sb = persist.tile([128, 4], F32, name="b_qk_sb")
    b_v_sb = persist.tile([1, E_V], F32R, name="b_v_sb")
    ones_sb = persist.tile([1, 128], F32R, name="ones_sb")
    qkT_sb = persist.tile([128, 4, S], F32R, name="qkT_sb")   # tiles 0-1: qT, 2-3: kT
    v_sb = persist.tile([128, S_TILES, HPC, HD + 1], BF16, name="v_sb")
    attnT_sb = persist.tile([128, 2, S], F32R, name="attnT_sb")

    # ---- input DMAs, ordered so the first attention chain's data lands
    # first: xT s-chunk 0 + the pair-0 q,k weight columns, then the rest ----
    dsl = lambda t: slice(t * 128, (t + 1) * 128)
    for t in range(D_TILES):
        nc.sync.dma_start(xT_sb[:, t, 0:512], xT[dsl(t), 0:512])
        nc.sync.dma_start(w_qk_sb[:, t, 0:384], w_qk[dsl(t), 0:384])  # q + k pair0 (+q pair1)
    nc.sync.dma_start(b_qk_sb[:], b_qk[:, :])
    nc.sync.dma_start(b_v_sb[:], b_v[:, :])
    nc.sync.dma_start(ones_sb[:], ones[0:1, 0:128])
    nc.sync.dma_start(v_sb[:, :, :, HD], ones16[:, 0:64].rearrange("p (s h) -> p s h", s=S_TILES))
    for t in range(D_TILES):
        nc.sync.dma_start(w_v_sb[:, t, :], w_v[dsl(t), :])
    for sc in range(1, I_CHUNKS):
        for t in range(D_TILES):
            nc.sync.dma_start(xT_sb[:, t, sc * 512:(sc + 1) * 512],
                              xT[dsl(t), sc * 512:(sc + 1) * 512])
    for t in range(D_TILES):
        nc.sync.dma_start(w_qk_sb[:, t, 384:512], w_qk[dsl(t), 384:512])  # k pair1
    for t in range(2):
        nc.sync.dma_start(w_o_sb[:, t, :], w_o[dsl(t), :])

    # ---- projection emitters ----
    def emit_qk(et, sc):
        psum = ps_q.tile([128, 512], F32, name="ps_qk", tag="psq")
        for d in range(D_TILES):
            nc.tensor.matmul(
                psum,
                (w_qk_sb[:, d, et * 128:(et + 1) * 128]),
                (xT_sb[:, d, sc * 512:(sc + 1) * 512]),
                start=(d == 0), stop=(d == D_TILES - 1),
            )
        nc.vector.tensor_scalar_add(
            qkT_sb[:, et, sc * 512:(sc + 1) * 512], psum, b_qk_sb[:, et:et + 1],
        )

    def emit_v():
        for st in range(S_TILES):
            psum = ps_q.tile([128, 512], F32, name="ps_v", tag="psq")[:, :E_V]
            for d in range(D_TILES):
                nc.tensor.matmul(
                    psum,
                    (xT_sb[:, d, st * 128:(st + 1) * 128]),
                    (w_v_sb[:, d, :]),
                    start=(d == 0), stop=False,
                )
            # bias via rank-1 ones matmul (K=1)
            nc.tensor.matmul(psum, (ones_sb[:, :]), (b_v_sb[:, :]), start=False, stop=True)
            nc.vector.tensor_copy(
                v_sb[:, st, :, 0:HD],
                psum.rearrange("p (h e) -> p h e", h=HPC),
            )

    def emit_attention(ic, pair):
        if True:
            isl = slice(ic * 512, (ic + 1) * 512)
            pvA = ps_pv.tile([HD + 1, 512], F32, name="pvA", tag="pv")
            pvB = ps_pv.tile([HD + 1, 512], F32, name="pvB", tag="pv")
            for j in range(J_TILES):
                jsl = slice(j * 128, (j + 1) * 128)
                psS = ps_s.tile([128, 1024], F32, name="psS", tag="pss")
                nc.tensor.matmul(
                    psS[:, 0:512],
                    (qkT_sb[0:64, 2 + pair, jsl]),
                    (qkT_sb[0:64, pair, isl]),
                    start=True, stop=True, tile_position=(0, 0),
                )
                nc.tensor.matmul(
                    psS[:, 512:1024],
                    (qkT_sb[64:128, 2 + pair, jsl]),
                    (qkT_sb[64:128, pair, isl]),
                    start=True, stop=True, tile_position=(64, 0),
                )
                e_t = epool.tile([128, 1024], BF16, name="e_t")
                nc.scalar.activation(e_t[:], psS[:], Exp, scale=0.125)
                nc.tensor.matmul(
                    pvA[:], (v_sb[:, j, 2 * pair, :]), (e_t[:, 0:512]),
                    start=(j == 0), stop=(j == J_TILES - 1),
                )
                nc.tensor.matmul(
                    pvB[:], (v_sb[:, j, 2 * pair + 1, :]), (e_t[:, 512:1024]),
                    start=(j == 0), stop=(j == J_TILES - 1),
                )
            for h_loc, pv in ((0, pvA), (1, pvB)):
                rec = rpool.tile([1, 512], F32, name="rec", tag="rec")
                nc.vector.reciprocal(rec[:], pv[HD:HD + 1, :])
                rb = rpool.tile([HD, 512], F32, name="rb", tag="rb")
                nc.gpsimd.partition_broadcast(rb[:], rec[:])
                nc.vector.tensor_mul(
                    attnT_sb[h_loc * 64:(h_loc + 1) * 64, pair, isl],
                    pv[0:HD, :],
                    rb[:],
                )

    def emit_final(ic):
        # final projection for the 4 s-tiles of this query chunk
        for st in range(ic * 4, ic * 4 + 4):
            ssl = slice(st * 128, (st + 1) * 128)
            for mc in range(2):
                msl = slice(mc * 512, (mc + 1) * 512)
                psF = ps_q.tile([128, 512], F32, name="ps_f", tag="psq")
                for kk in range(2):
                    nc.tensor.matmul(
                        psF,
                        (attnT_sb[:, kk, ssl]),
                        (w_o_sb[:, kk, msl]),
                        start=(kk == 0), stop=(kk == 1),
                    )
                o_t = spool.tile([128, 512], F32, name="o_t")
                nc.vector.tensor_copy(o_t[:], psF)
                nc.sync.dma_start(out[ssl, msl], o_t[:])

    # ---- emission order: pair-0 attention starts as early as possible so
    # the Activation engine (softmax exp, the co-bottleneck) fills while the
    # PE is still on projections ----
    # pair-0 q,k projections sc-major so the first attention chain's two
    # bias-adds are the first two DVE ops
    for sc in range(I_CHUNKS):
        emit_qk(2, sc)    # k pair 0
        emit_qk(0, sc)    # q pair 0
    emit_v()
    emit_attention(0, 0)
    for sc in range(I_CHUNKS):
        emit_qk(3, sc)    # k pair 1 (PE filler while ACT chews on ic0/pair0)
        emit_qk(1, sc)    # q pair 1
    emit_attention(0, 1)
    for ic in range(1, I_CHUNKS):
        emit_attention(ic, 0)
        emit_final(ic - 1)   # previous chunk's projection: PE filler
        emit_attention(ic, 1)
    emit_final(I_CHUNKS - 1)


_CACHE = {}


def _get_nc(reps=1):
    key = ("nc", reps)
    if key not in _CACHE:
        _CACHE[key] = _build_nc(reps)
    return _CACHE[key]


def _get_runner(reps=1):
    """Build (once) a jitted shard_map executable over the 8 cores.

    Mirrors bass2jax.run_bass_via_pjrt but caches the jitted function so
    repeat kernel() calls and benchmarking skip retrace/recompile.
    """
    if ("runner", reps) in _CACHE:
        return _CACHE[("runner", reps)]
    import jax
    import jax.numpy as jnp
    from jax.sharding import Mesh, PartitionSpec
    from jax.experimental.shard_map import shard_map
    from concourse import bass2jax

    nc = _get_nc(reps)
    bass2jax.install_neuronx_cc_hook()

    partition_name = nc.partition_id_tensor.name if nc.partition_id_tensor else None
    in_names, out_names, out_avals = [], [], []
    for alloc in nc.m.functions[0].allocations:
        if not isinstance(alloc, mybir.MemoryLocationSet):
            continue
        name = alloc.memorylocations[0].name
        if alloc.kind == "ExternalInput":
            if name != partition_name:
                in_names.append(name)
        elif alloc.kind == "ExternalOutput":
            shape = tuple(alloc.tensor_shape)
            dtype = mybir.dt.np(alloc.dtype)
            out_names.append(name)
            out_avals.append(jax.core.ShapedArray(shape, dtype))
    n_params = len(in_names)
    n_outs = len(out_avals)
    all_names = in_names + out_names
    if partition_name is not None:
        all_names = all_names + [partition_name]
    donate = tuple(range(n_params, n_params + n_outs))

    def _jit_body(*args):
        operands = list(args)
        if partition_name is not None:
            operands.append(bass2jax.partition_id_tensor())
        outs = bass2jax._bass_exec_p.bind(
            *operands,
            out_avals=tuple(out_avals),
            in_names=tuple(all_names),
            out_names=tuple(out_names),
            lowering_input_output_aliases=(),
            sim_require_finite=True,
            sim_require_nnan=True,
            nc=nc,
        )
        return tuple(outs)

    devices = jax.devices()[:NCORES]
    mesh = Mesh(np.asarray(devices), ("core",))
    sharded = jax.jit(
        shard_map(
            _jit_body, mesh=mesh,
            in_specs=(PartitionSpec("core"),) * (n_params + n_outs),
            out_specs=(PartitionSpec("core"),) * n_outs,
            check_rep=False,
        ),
        donate_argnums=donate, keep_unused=True,
    )

    from jax.sharding import NamedSharding
    core_sharding = NamedSharding(mesh, PartitionSpec("core"))

    @partial(jax.jit, out_shardings=core_sharding)
    def _zeros():
        return tuple(
            jnp.zeros((NCORES * a.shape[0],) + a.shape[1:], a.dtype)
            for a in out_avals)

    def run(in_maps, device_arrays=None, timeit=False):
        import time as _time
        if device_arrays is None:
            concat_in = [
                np.concatenate([np.asarray(m[name]) for m in in_maps], axis=0)
                for name in in_names]
            device_arrays = [jax.device_put(a, core_sharding) for a in concat_in]
        zs = jax.block_until_ready(_zeros())
        t0 = _time.perf_counter()
        out_arrs = jax.block_until_ready(sharded(*device_arrays, *zs))
        dt = _time.perf_counter() - t0
        results = [
            {name: np.asarray(out_arrs[i]).reshape(NCORES, *out_avals[i].shape)[c]
             for i, name in enumerate(out_names)}
            for c in range(NCORES)]
        if timeit:
            return results, dt, device_arrays
        return results

    def bench(in_maps, iters=10, batches=3):
        """Pipelined timing: dispatch `iters` executions back-to-back and
        block once, amortizing the per-dispatch RPC latency.  Returns the
        min per-iter average across `batches` batches."""
        import time as _time
        concat_in = [
            np.concatenate([np.asarray(m[name]) for m in in_maps], axis=0)
            for name in in_names]
        device_arrays = [jax.device_put(a, core_sharding) for a in concat_in]
        best = None
        for b in range(batches):
            all_zs = [jax.block_until_ready(_zeros()) for _ in range(iters + 1)]
            jax.block_until_ready(sharded(*device_arrays, *all_zs[0]))
            t0 = _time.perf_counter()
            outs = [sharded(*device_arrays, *all_zs[1 + i]) for i in range(iters)]
            jax.block_until_ready(outs)
            dt = (_time.perf_counter() - t0) / iters
            best = dt if best is None else min(best, dt)
        return best

    _CACHE[("bench", reps)] = bench
    _CACHE[("runner", reps)] = run
    return run


def _core_inputs(x, w_qkv, b_qkv, w_out):
    """Host-side sharding: returns the 8 per-core input dicts."""
    in_maps = []
    for c in range(NCORES):
        b, g = divmod(c, 4)
        e0 = g * HPC * HD  # first column of this core's head group
        q_cols = slice(e0, e0 + E_V)
        k_cols = slice(D + e0, D + e0 + E_V)
        v_cols = slice(2 * D + e0, 2 * D + e0 + E_V)
        w_qk_c = np.ascontiguousarray(
            np.concatenate([w_qkv[:, q_cols], w_qkv[:, k_cols]], axis=1))
        b_qk_c = np.ascontiguousarray(
            np.concatenate([b_qkv[q_cols], b_qkv[k_cols]]).reshape(4, 128).T)
        in_maps.append({
            "xT": np.ascontiguousarray(x[b].T),
            "w_qk": w_qk_c,
            "w_v": np.ascontiguousarray(w_qkv[:, v_cols]),
            "w_o": np.ascontiguousarray(w_out[e0:e0 + E_V, :]),
            "b_qk": b_qk_c,
            "b_v": np.ascontiguousarray(b_qkv[v_cols]).reshape(1, E_V),
            "ones": np.ones((128, 128), dtype=np.float32),
            "ones16": np.ones((128, 64), dtype=ml_dtypes.bfloat16),
        })
    return in_maps


def kernel(x, w_qkv, b_qkv, w_out, b_out):
    x = np.asarray(x, dtype=np.float32)
    w_qkv = np.asarray(w_qkv, dtype=np.float32)
    b_qkv = np.asarray(b_qkv, dtype=np.float32)
    w_out = np.asarray(w_out, dtype=np.float32)
    b_out = np.asarray(b_out, dtype=np.float32)

    run = _get_runner()
    in_maps = _core_inputs(x, w_qkv, b_qkv, w_out)
    results = run(in_maps)
    partials = np.stack([results[c]["out"] for c in range(NCORES)])
    full = partials.reshape(B, 4, S, D).sum(axis=1) + b_out
    return full.astype(np.float32)


# revision 24
# speedup vs baseline: 1.1652x; 1.1302x over previous
"""Trainium2 Bass kernel for nn_AttentionBlock (B=2, S=2048, D=1024, H=16).

Sharding: 8 cores = data-parallel over batch (2) x tensor-parallel over
head groups (4 heads per core).  Each core computes its 4 heads'
attention plus its slice of the qkv / out projections; the host sums the
4 per-batch partial outputs and adds b_out.

Per-core layout plan (all matmuls in float32r, N>=256):
  - host passes x[b].T so the contraction dim (d) is the partition dim
  - q,k computed transposed [e, s]; v computed natural [s, hd]
  - S^T[j,i] = k_h q_h^T, two heads packed in the PE array (row groups)
  - exp on ScalarE straight out of PSUM (scale = 1/8 folded into exp)
  - PV matmul with stationary [v_h | ones] -> unnormalized out^T plus the
    softmax row-sum in PSUM row 64, in one pass over E
  - normalize: reciprocal + gpsimd partition_broadcast + DVE multiply
  - final projection consumes the transposed attention output directly
"""

from contextlib import ExitStack
from functools import partial

import ml_dtypes
import numpy as np

import concourse.bass as bass
import concourse.tile as tile
from concourse import bacc, mybir
from concourse import bass_utils

B, S, D = 2, 2048, 1024
HD = 64          # head dim
HPC = 4          # heads per core
E_QK = 512       # q+k columns per core (2 * HPC * HD)
E_V = 256        # v columns per core
NCORES = 8

F32 = mybir.dt.float32
F32R = mybir.dt.float32r
BF16 = mybir.dt.bfloat16

S_TILES = S // 128       # 16
D_TILES = D // 128       # 8
I_CHUNKS = S // 512      # 4 query chunks
J_TILES = S // 128       # 16 key tiles


def _build_nc(reps=1):
    nc = bacc.Bacc("TRN2", target_bir_lowering=False, debug=False, num_devices=NCORES)

    xT = nc.dram_tensor("xT", [D, S], F32R, kind="ExternalInput")
    w_qk = nc.dram_tensor("w_qk", [D, E_QK], F32R, kind="ExternalInput")
    w_v = nc.dram_tensor("w_v", [D, E_V], F32R, kind="ExternalInput")
    w_o = nc.dram_tensor("w_o", [E_V, D], F32R, kind="ExternalInput")
    b_qk = nc.dram_tensor("b_qk", [128, 4], F32, kind="ExternalInput")
    b_v = nc.dram_tensor("b_v", [1, E_V], F32R, kind="ExternalInput")
    ones = nc.dram_tensor("ones", [128, 128], F32R, kind="ExternalInput")
    out = nc.dram_tensor("out", [S, D], F32, kind="ExternalOutput")

    with tile.TileContext(nc) as tc, ExitStack() as ctx:
        if reps == 1:
            _body(ctx, tc, xT.ap(), w_qk.ap(), w_v.ap(), w_o.ap(), b_qk.ap(), b_v.ap(), ones.ap(), out.ap())
        else:
            with tc.For_i(0, reps) as _i:
                with ExitStack() as ictx:
                    _body(ictx, tc, xT.ap(), w_qk.ap(), w_v.ap(), w_o.ap(), b_qk.ap(), b_v.ap(), ones.ap(), out.ap())
    nc.compile()
    return nc


def _body(ctx, tc, xT, w_qk, w_v, w_o, b_qk, b_v, ones, out):
    nc = tc.nc
    Exp = mybir.ActivationFunctionType.Exp

    persist = ctx.enter_context(tc.tile_pool(name="persist", bufs=1))
    ps_s = ctx.enter_context(tc.tile_pool(name="ps_s", bufs=2, space="PSUM"))
    ps_q = ctx.enter_context(tc.tile_pool(name="ps_q", bufs=2, space="PSUM"))
    ps_pv = ctx.enter_context(tc.tile_pool(name="ps_pv", bufs=2, space="PSUM"))
    epool = ctx.enter_context(tc.tile_pool(name="epool", bufs=6))
    spool = ctx.enter_context(tc.tile_pool(name="spool", bufs=3))
    rpool = ctx.enter_context(tc.tile_pool(name="rpool", bufs=4))

    # ---- persistent SBUF tensors ----
    xT_sb = persist.tile([128, D_TILES, S], F32R, name="xT_sb")
    w_qk_sb = persist.tile([128, D_TILES, E_QK], F32R, name="w_qk_sb")
    w_v_sb = persist.tile([128, D_TILES, E_V], F32R, name="w_v_sb")
    w_o_sb = persist.tile([128, 2, D], F32R, name="w_o_sb")
    b_qk_sb = persist.tile([128, 4], F32, name="b_qk_sb")
    b_v_sb = persist.tile([1, E_V], F32R, name="b_v_sb")
    ones_sb = persist.tile([1, 128], F32R, name="ones_sb")
    qkT_sb = persist.tile([128, 4, S], F32R, name="qkT_sb")   # tiles 0-1: qT, 2-3: kT
    v_sb = persist.tile([128, S_TILES, HPC, HD + 1], F32R, name="v_sb")
    attnT_sb = persist.tile([128, 2, S], F32R, name="attnT_sb")

    # ---- input DMAs, ordered so the first attention chain's data lands
    # first: xT s-chunk 0 + the pair-0 q,k weight columns, then the rest ----
    dsl = lambda t: slice(t * 128, (t + 1) * 128)
    for t in range(D_TILES):
        nc.sync.dma_start(xT_sb[:, t, 0:512], xT[dsl(t), 0:512])
        nc.sync.dma_start(w_qk_sb[:, t, 0:384], w_qk[dsl(t), 0:384])  # q + k pair0 (+q pair1)
    nc.sync.dma_start(b_qk_sb[:], b_qk[:, :])
    nc.sync.dma_start(b_v_sb[:], b_v[:, :])
    nc.sync.dma_start(ones_sb[:], ones[0:1, 0:128])
    nc.sync.dma_start(v_sb[:, :, :, HD], ones[:, 0:64].rearrange("p (s h) -> p s h", s=S_TILES))
    for t in range(D_TILES):
        nc.sync.dma_start(w_v_sb[:, t, :], w_v[dsl(t), :])
    for sc in range(1, I_CHUNKS):
        for t in range(D_TILES):
            nc.sync.dma_start(xT_sb[:, t, sc * 512:(sc + 1) * 512],
                              xT[dsl(t), sc * 512:(sc + 1) * 512])
    for t in range(D_TILES):
        nc.sync.dma_start(w_qk_sb[:, t, 384:512], w_qk[dsl(t), 384:512])  # k pair1
    for t in range(2):
        nc.sync.dma_start(w_o_sb[:, t, :], w_o[dsl(t), :])

    # ---- projection emitters ----
    def emit_qk(et, sc):
        psum = ps_q.tile([128, 512], F32, name="ps_qk", tag="psq")
        for d in range(D_TILES):
            nc.tensor.matmul(
                psum,
                (w_qk_sb[:, d, et * 128:(et + 1) * 128]),
                (xT_sb[:, d, sc * 512:(sc + 1) * 512]),
                start=(d == 0), stop=(d == D_TILES - 1),
            )
        nc.vector.tensor_scalar_add(
            qkT_sb[:, et, sc * 512:(sc + 1) * 512], psum, b_qk_sb[:, et:et + 1],
        )

    def emit_v():
        for st in range(S_TILES):
            psum = ps_q.tile([128, 512], F32, name="ps_v", tag="psq")[:, :E_V]
            for d in range(D_TILES):
                nc.tensor.matmul(
                    psum,
                    (xT_sb[:, d, st * 128:(st + 1) * 128]),
                    (w_v_sb[:, d, :]),
                    start=(d == 0), stop=False,
                )
            # bias via rank-1 ones matmul (K=1)
            nc.tensor.matmul(psum, (ones_sb[:, :]), (b_v_sb[:, :]), start=False, stop=True)
            nc.vector.tensor_copy(
                v_sb[:, st, :, 0:HD],
                psum.rearrange("p (h e) -> p h e", h=HPC),
            )

    def emit_attention(ic, pair):
        if True:
            isl = slice(ic * 512, (ic + 1) * 512)
            pvA = ps_pv.tile([HD + 1, 512], F32, name="pvA", tag="pv")
            pvB = ps_pv.tile([HD + 1, 512], F32, name="pvB", tag="pv")
            for j in range(J_TILES):
                jsl = slice(j * 128, (j + 1) * 128)
                psS = ps_s.tile([128, 1024], F32, name="psS", tag="pss")
                nc.tensor.matmul(
                    psS[:, 0:512],
                    (qkT_sb[0:64, 2 + pair, jsl]),
                    (qkT_sb[0:64, pair, isl]),
                    start=True, stop=True, tile_position=(0, 0),
                )
                nc.tensor.matmul(
                    psS[:, 512:1024],
                    (qkT_sb[64:128, 2 + pair, jsl]),
                    (qkT_sb[64:128, pair, isl]),
                    start=True, stop=True, tile_position=(64, 0),
                )
                e_t = epool.tile([128, 1024], F32R, name="e_t")
                nc.scalar.activation(e_t[:], psS[:], Exp, scale=0.125)
                nc.tensor.matmul(
                    pvA[:], (v_sb[:, j, 2 * pair, :]), (e_t[:, 0:512]),
                    start=(j == 0), stop=(j == J_TILES - 1),
                )
                nc.tensor.matmul(
                    pvB[:], (v_sb[:, j, 2 * pair + 1, :]), (e_t[:, 512:1024]),
                    start=(j == 0), stop=(j == J_TILES - 1),
                )
            for h_loc, pv in ((0, pvA), (1, pvB)):
                rec = rpool.tile([1, 512], F32, name="rec", tag="rec")
                nc.vector.reciprocal(rec[:], pv[HD:HD + 1, :])
                rb = rpool.tile([HD, 512], F32, name="rb", tag="rb")
                nc.gpsimd.partition_broadcast(rb[:], rec[:])
                nc.vector.tensor_mul(
                    attnT_sb[h_loc * 64:(h_loc + 1) * 64, pair, isl],
                    pv[0:HD, :],
                    rb[:],
                )

    def emit_final(ic):
        # final projection for the 4 s-tiles of this query chunk
        for st in range(ic * 4, ic * 4 + 4):
            ssl = slice(st * 128, (st + 1) * 128)
            for mc in range(2):
                msl = slice(mc * 512, (mc + 1) * 512)
                psF = ps_q.tile([128, 512], F32, name="ps_f", tag="psq")
                for kk in range(2):
                    nc.tensor.matmul(
                        psF,
                        (attnT_sb[:, kk, ssl]),
                        (w_o_sb[:, kk, msl]),
                        start=(kk == 0), stop=(kk == 1),
                    )
                o_t = spool.tile([128, 512], F32, name="o_t")
                nc.vector.tensor_copy(o_t[:], psF)
                nc.sync.dma_start(out[ssl, msl], o_t[:])

    # ---- emission order: pair-0 attention starts as early as possible so
    # the Activation engine (softmax exp, the co-bottleneck) fills while the
    # PE is still on projections ----
    # pair-0 q,k projections sc-major so the first attention chain's two
    # bias-adds are the first two DVE ops
    for sc in range(I_CHUNKS):
        emit_qk(2, sc)    # k pair 0
        emit_qk(0, sc)    # q pair 0
    emit_v()
    emit_attention(0, 0)
    for sc in range(I_CHUNKS):
        emit_qk(3, sc)    # k pair 1 (PE filler while ACT chews on ic0/pair0)
        emit_qk(1, sc)    # q pair 1
    emit_attention(0, 1)
    for ic in range(1, I_CHUNKS):
        emit_attention(ic, 0)
        emit_final(ic - 1)   # previous chunk's projection: PE filler
        emit_attention(ic, 1)
    emit_final(I_CHUNKS - 1)


_CACHE = {}


def _get_nc(reps=1):
    key = ("nc", reps)
    if key not in _CACHE:
        _CACHE[key] = _build_nc(reps)
    return _CACHE[key]


def _get_runner(reps=1):
    """Build (once) a jitted shard_map executable over the 8 cores.

    Mirrors bass2jax.run_bass_via_pjrt but caches the jitted function so
    repeat kernel() calls and benchmarking skip retrace/recompile.
    """
    if ("runner", reps) in _CACHE:
        return _CACHE[("runner", reps)]
    import jax
    import jax.numpy as jnp
    from jax.sharding import Mesh, PartitionSpec
    from jax.experimental.shard_map import shard_map
    from concourse import bass2jax

    nc = _get_nc(reps)
    bass2jax.install_neuronx_cc_hook()

    partition_name = nc.partition_id_tensor.name if nc.partition_id_tensor else None
    in_names, out_names, out_avals = [], [], []
    for alloc in nc.m.functions[0].allocations:
        if not isinstance(alloc, mybir.MemoryLocationSet):
            continue
        name = alloc.memorylocations[0].name
        if alloc.kind == "ExternalInput":
            if name != partition_name:
                in_names.append(name)
        elif alloc.kind == "ExternalOutput":
            shape = tuple(alloc.tensor_shape)
            dtype = mybir.dt.np(alloc.dtype)
            out_names.append(name)
            out_avals.append(jax.core.ShapedArray(shape, dtype))
    n_params = len(in_names)
    n_outs = len(out_avals)
    all_names = in_names + out_names
    if partition_name is not None:
        all_names = all_names + [partition_name]
    donate = tuple(range(n_params, n_params + n_outs))

    def _jit_body(*args):
        operands = list(args)
        if partition_name is not None:
            operands.append(bass2jax.partition_id_tensor())
        outs = bass2jax._bass_exec_p.bind(
            *operands,
            out_avals=tuple(out_avals),
            in_names=tuple(all_names),
            out_names=tuple(out_names),
            lowering_input_output_aliases=(),
            sim_require_finite=True,
            sim_require_nnan=True,
            nc=nc,
        )
        return tuple(outs)

    devices = jax.devices()[:NCORES]
    mesh = Mesh(np.asarray(devices), ("core",))
    sharded = jax.jit(
        shard_map(
            _jit_body, mesh=mesh,
            in_specs=(PartitionSpec("core"),) * (n_params + n_outs),
            out_specs=(PartitionSpec("core"),) * n_outs,
            check_rep=False,
        ),
        donate_argnums=donate, keep_unused=True,
    )

    from jax.sharding import NamedSharding
    core_sharding = NamedSharding(mesh, PartitionSpec("core"))

    @partial(jax.jit, out_shardings=core_sharding)
    def _zeros():
        return tuple(
            jnp.zeros((NCORES * a.shape[0],) + a.shape[1:], a.dtype)
            for a in out_avals)

    def run(in_maps, device_arrays=None, timeit=False):
        import time as _time
        if device_arrays is None:
            concat_in = [
                np.concatenate([np.asarray(m[name]) for m in in_maps], axis=0)
                for name in in_names]
            device_arrays = [jax.device_put(a, core_sharding) for a in concat_in]
        zs = jax.block_until_ready(_zeros())
        t0 = _time.perf_counter()
        out_arrs = jax.block_until_ready(sharded(*device_arrays, *zs))
        dt = _time.perf_counter() - t0
        results = [
            {name: np.asarray(out_arrs[i]).reshape(NCORES, *out_avals[i].shape)[c]
             for i, name in enumerate(out_names)}
            for c in range(NCORES)]
        if timeit:
            return results, dt, device_arrays
        return results

    def bench(in_maps, iters=10, batches=3):
        """Pipelined timing: dispatch `iters` executions back-to-back and
        block once, amortizing the per-dispatch RPC latency.  Returns the
        min per-iter average across `batches` batches."""
        import time as _time
        concat_in = [
            np.concatenate([np.asarray(m[name]) for m in in_maps], axis=0)
            for name in in_names]
        device_arrays = [jax.device_put(a, core_sharding) for a in concat_in]
        best = None
        for b in range(batches):
            all_zs = [jax.block_until_ready(_zeros()) for _ in range(iters + 1)]
            jax.block_until_ready(sharded(*device_arrays, *all_zs[0]))
            t0 = _time.perf_counter()
            outs = [sharded(*device_arrays, *all_zs[1 + i]) for i in range(iters)]
            jax.block_until_ready(outs)
            dt = (_time.perf_counter() - t0) / iters
            best = dt if best is None else min(best, dt)
        return best

    _CACHE[("bench", reps)] = bench
    _CACHE[("runner", reps)] = run
    return run


def _core_inputs(x, w_qkv, b_qkv, w_out):
    """Host-side sharding: returns the 8 per-core input dicts."""
    in_maps = []
    for c in range(NCORES):
        b, g = divmod(c, 4)
        e0 = g * HPC * HD  # first column of this core's head group
        q_cols = slice(e0, e0 + E_V)
        k_cols = slice(D + e0, D + e0 + E_V)
        v_cols = slice(2 * D + e0, 2 * D + e0 + E_V)
        w_qk_c = np.ascontiguousarray(
            np.concatenate([w_qkv[:, q_cols], w_qkv[:, k_cols]], axis=1))
        b_qk_c = np.ascontiguousarray(
            np.concatenate([b_qkv[q_cols], b_qkv[k_cols]]).reshape(4, 128).T)
        in_maps.append({
            "xT": np.ascontiguousarray(x[b].T),
            "w_qk": w_qk_c,
            "w_v": np.ascontiguousarray(w_qkv[:, v_cols]),
            "w_o": np.ascontiguousarray(w_out[e0:e0 + E_V, :]),
            "b_qk": b_qk_c,
            "b_v": np.ascontiguousarray(b_qkv[v_cols]).reshape(1, E_V),
            "ones": np.ones((128, 128), dtype=np.float32),
        })
    return in_maps


def kernel(x, w_qkv, b_qkv, w_out, b_out):
    x = np.asarray(x, dtype=np.float32)
    w_qkv = np.asarray(w_qkv, dtype=np.float32)
    b_qkv = np.asarray(b_qkv, dtype=np.float32)
    w_out = np.asarray(w_out, dtype=np.float32)
    b_out = np.asarray(b_out, dtype=np.float32)

    run = _get_runner()
    in_maps = _core_inputs(x, w_qkv, b_qkv, w_out)
    results = run(in_maps)
    partials = np.stack([results[c]["out"] for c in range(NCORES)])
    full = partials.reshape(B, 4, S, D).sum(axis=1) + b_out
    return full.astype(np.float32)
